# revision 4
# baseline (speedup 1.0000x reference)
"""Trainium2 Bass kernel for nn_BilinearDense.

Math:
  W = (z @ W_kernel + W_bias).reshape(B, OD, XD)      # per-sample matrix
  b = z @ b_kernel + b_bias                            # per-sample bias
  out[b,o] = sum_i W[b,o,i] x[b,i] + b[b,o]
           = sum_{k,i} z[b,k] x[b,i] W_kernel[k, o*XD+i]  (+ bias terms)

Strategy (8 NeuronCores, batch-sharded, 512 samples/core):
  The double contraction over (k,i) is done as ONE long PE-matmul
  accumulation per core:
     outT[o, b] = sum_{ki} Wt[ki, o] * PT[ki, b]
  where Wt[(k,i), o] = W_kernel[k, o*XD+i] (host-side transform, bf16)
  and   PT[(k,i), b] = z[b,k] * x[b,i] is built on-chip by the Vector
  engine as bf16 SBUF tensor_tensor products (2x perf mode):
     PT tile[i-half, (k, b)] = x.T[i-half, b] * broadcast(z.T[k, b])
  The z-broadcast tiles are pre-replicated on the host (zrep) so DMA
  reads are fully contiguous.  Bias terms are folded into the same PSUM
  accumulation with a few extra tiny matmuls.  PE does 1024 matmuls of
  [K=128] x [M=128, N=512] bf16 per core (~17.2 GFLOP/core).
"""

import numpy as np
import ml_dtypes

B, XD, ZD, OD = 4096, 256, 256, 256
NCORES = 8
BS = B // NCORES  # batch shard per core
KG = 8            # k's per group (z-replica slab granularity)
NG = ZD // KG     # 32 groups
P = 128

BF = ml_dtypes.bfloat16

_prog_cache = {}


def _build_program():
    if "nc" in _prog_cache:
        return _prog_cache["nc"], _prog_cache["dout_name"]

    import concourse.bass as bass
    import concourse.tile as tile
    from concourse import bacc, mybir

    bf16 = mybir.dt.bfloat16
    f32 = mybir.dt.float32

    nc = bacc.Bacc(
        "TRN2", target_bir_lowering=False, debug=False, num_devices=NCORES
    )

    d_xrep = nc.dram_tensor("xrep", [XD, KG * BS], bf16, kind="ExternalInput").ap()
    d_zrep = nc.dram_tensor("zrep", [NG, P, KG * BS], bf16, kind="ExternalInput").ap()
    d_wt = nc.dram_tensor("wtg", [NG, 2, P, KG * OD], bf16, kind="ExternalInput").ap()
    d_zt = nc.dram_tensor("zt", [ZD, BS], bf16, kind="ExternalInput").ap()
    d_bk = nc.dram_tensor("bk", [ZD, OD], bf16, kind="ExternalInput").ap()
    d_wbt = nc.dram_tensor("wbt", [XD, OD], bf16, kind="ExternalInput").ap()
    d_bb = nc.dram_tensor("bb", [1, OD], bf16, kind="ExternalInput").ap()
    d_ones = nc.dram_tensor("ones", [1, BS], bf16, kind="ExternalInput").ap()
    d_out = nc.dram_tensor("outT", [OD, BS], f32, kind="ExternalOutput").ap()

    with tile.TileContext(nc) as tc:
        with (
            tc.tile_pool(name="const", bufs=1) as cpool,
            tc.tile_pool(name="zslab", bufs=4) as zpool,
            tc.tile_pool(name="wslab", bufs=6) as wpool,
            tc.tile_pool(name="pt", bufs=4) as ptpool,
            tc.tile_pool(name="outp", bufs=1) as opool,
            tc.tile_pool(name="psum", bufs=1, space="PSUM") as psum,
        ):
            # --- latency-critical first loads: split into quarters so the
            # first matmuls can start as soon as the first chunk lands ---
            NQ = 4
            QW = KG * BS // NQ  # 1024
            xr = []
            for ih in range(2):
                t = cpool.tile([P, KG * BS], bf16, tag=f"xr{ih}", name=f"xr{ih}")
                if ih == 0:
                    for q in range(NQ):
                        nc.sync.dma_start(
                            t[:, q * QW : (q + 1) * QW],
                            d_xrep[0:P, q * QW : (q + 1) * QW],
                        )
                else:
                    nc.sync.dma_start(t[:], d_xrep[P : 2 * P, :])
                xr.append(t)
            zs0 = zpool.tile([P, KG * BS], bf16, tag="zs", name="zs0")
            for q in range(NQ):
                nc.sync.dma_start(
                    zs0[:, q * QW : (q + 1) * QW], d_zrep[0][:, q * QW : (q + 1) * QW]
                )
            ws00 = wpool.tile([P, KG * OD], bf16, tag="ws", name="ws00")
            nc.sync.dma_start(ws00[:], d_wt[0][0])
            ws01 = wpool.tile([P, KG * OD], bf16, tag="ws", name="ws01")
            nc.sync.dma_start(ws01[:], d_wt[0][1])

            # --- small bias constants (off the critical path) ---
            ztt, bkt, wbtt = [], [], []
            for kc in range(2):
                t = cpool.tile([P, BS], bf16, tag=f"zt{kc}", name=f"zt{kc}")
                nc.sync.dma_start(t[:], d_zt[kc * P : (kc + 1) * P, :])
                ztt.append(t)
                t = cpool.tile([P, OD], bf16, tag=f"bk{kc}", name=f"bk{kc}")
                nc.sync.dma_start(t[:], d_bk[kc * P : (kc + 1) * P, :])
                bkt.append(t)
                t = cpool.tile([P, OD], bf16, tag=f"wbt{kc}", name=f"wbt{kc}")
                nc.sync.dma_start(t[:], d_wbt[kc * P : (kc + 1) * P, :])
                wbtt.append(t)
            bbt = cpool.tile([1, OD], bf16, tag="bb")
            nc.sync.dma_start(bbt[:], d_bb[:])
            onest = cpool.tile([1, BS], bf16, tag="ones")
            nc.sync.dma_start(onest[:], d_ones[:])

            pso = [psum.tile([P, BS], f32, tag=f"ps{oh}", name=f"ps{oh}") for oh in range(2)]

            started = [False, False]

            def mm(oh, lhsT, rhs, stop=False):
                nc.tensor.matmul(
                    pso[oh][:], lhsT, rhs, start=not started[oh], stop=stop
                )
                started[oh] = True

            # --- main accumulation: 32 groups x 2 i-halves x 8 k x 2 o-halves ---
            for g in range(NG):
                if g == 0:
                    zs = zs0
                else:
                    zs = zpool.tile([P, KG * BS], bf16, tag="zs")
                    nc.sync.dma_start(zs[:], d_zrep[g])
                for ih in range(2):
                    if g == 0 and ih == 0:
                        ws = ws00
                    elif g == 0 and ih == 1:
                        ws = ws01
                    else:
                        ws = wpool.tile([P, KG * OD], bf16, tag="ws")
                        nc.sync.dma_start(ws[:], d_wt[g][ih])
                    pt = ptpool.tile([P, KG * BS], bf16, tag="pt")
                    if g == 0 and ih == 0:
                        # chunked multiply so matmuls start on first quarter
                        for q in range(NQ):
                            nc.vector.tensor_mul(
                                pt[:, q * QW : (q + 1) * QW],
                                xr[0][:, q * QW : (q + 1) * QW],
                                zs[:, q * QW : (q + 1) * QW],
                            )
                    else:
                        nc.vector.tensor_mul(pt[:], xr[ih][:], zs[:])
                    last_grp = (g == NG - 1) and (ih == 1)
                    if last_grp:
                        # o-half-major order so bank 0 finishes early and
                        # its drain overlaps bank 1's tail matmuls
                        for oh in range(2):
                            for kl in range(KG):
                                mm(
                                    oh,
                                    ws[:, kl * OD + oh * P : kl * OD + oh * P + P],
                                    pt[:, kl * BS : (kl + 1) * BS],
                                    stop=(kl == KG - 1),
                                )
                            ot = opool.tile(
                                [P, BS], f32, tag=f"ot{oh}", name=f"ot{oh}"
                            )
                            if oh == 0:
                                nc.vector.tensor_copy(ot[:], pso[oh][:])
                            else:
                                nc.scalar.copy(ot[:], pso[oh][:])
                            nc.sync.dma_start(d_out[oh * P : (oh + 1) * P, :], ot[:])
                    else:
                        for kl in range(KG):
                            for oh in range(2):
                                mm(
                                    oh,
                                    ws[:, kl * OD + oh * P : kl * OD + oh * P + P],
                                    pt[:, kl * BS : (kl + 1) * BS],
                                )
                if g == 0:
                    # bias matmuls, placed while PE is warm and all const
                    # DMAs have long landed
                    for oh in range(2):
                        osl = slice(oh * P, (oh + 1) * P)
                        mm(oh, bkt[0][:, osl], ztt[0][:])
                        mm(oh, bkt[1][:, osl], ztt[1][:])
                        mm(oh, wbtt[0][:, osl], xr[0][:, 0:BS])
                        mm(oh, wbtt[1][:, osl], xr[1][:, 0:BS])
                        mm(oh, bbt[:, osl], onest[:])

    nc.compile()

    _prog_cache["nc"] = nc
    _prog_cache["dout_name"] = "outT"
    return nc, "outT"


def _prep_inputs(x, z, W_kernel, W_bias, b_kernel, b_bias):
    x = np.asarray(x, dtype=np.float32)
    z = np.asarray(z, dtype=np.float32)
    W_kernel = np.asarray(W_kernel, dtype=np.float32)
    W_bias = np.asarray(W_bias, dtype=np.float32)
    b_kernel = np.asarray(b_kernel, dtype=np.float32)
    b_bias = np.asarray(b_bias, dtype=np.float32)

    # Wt[(k,i), o] = W_kernel[k, o*XD+i]
    Wt = W_kernel.reshape(ZD, OD, XD).transpose(0, 2, 1).astype(BF)  # [k, i, o]
    # group for contiguous DMA slabs: [g, ih, p, kl*OD+o]
    wtg = np.ascontiguousarray(
        Wt.reshape(NG, KG, 2, P, OD).transpose(0, 2, 3, 1, 4)
    ).reshape(NG, 2, P, KG * OD)

    bk = b_kernel.astype(BF)  # [ZD, OD]
    wbt = np.ascontiguousarray(W_bias.reshape(OD, XD).T).astype(BF)  # [XD, OD]
    bb = b_bias.reshape(1, OD).astype(BF)
    ones = np.ones((1, BS), dtype=BF)

    in_maps = []
    for c in range(NCORES):
        xs = x[c * BS : (c + 1) * BS]  # [BS, XD]
        zsh = z[c * BS : (c + 1) * BS]  # [BS, ZD]
        xT = np.ascontiguousarray(xs.T).astype(BF)  # [XD, BS]
        zT = np.ascontiguousarray(zsh.T).astype(BF)  # [ZD, BS]
        xrep = np.ascontiguousarray(np.tile(xT, (1, KG)))  # [XD, KG*BS]
        zflat = zT.reshape(NG, KG * BS)
        zrep = np.ascontiguousarray(
            np.broadcast_to(zflat[:, None, :], (NG, P, KG * BS))
        )
        in_maps.append(
            {
                "xrep": xrep,
                "zrep": zrep,
                "wtg": wtg,
                "zt": zT,
                "bk": bk,
                "wbt": wbt,
                "bb": bb,
                "ones": ones,
            }
        )
    return in_maps


def kernel_run(inputs, trace=False, trace_kwargs=None):
    """Run on hardware; returns (out [B,OD] float32, BassKernelResults)."""
    import concourse.bass_utils as bass_utils

    nc, out_name = _build_program()
    in_maps = _prep_inputs(**inputs)
    res = bass_utils.run_bass_kernel_spmd(
        nc,
        in_maps,
        core_ids=list(range(NCORES)),
        trace=trace,
        **(trace_kwargs or {}),
    )
    out = np.empty((B, OD), dtype=np.float32)
    for c in range(NCORES):
        out[c * BS : (c + 1) * BS, :] = res.results[c][out_name].T
    return out, res


def kernel(x, z, W_kernel, W_bias, b_kernel, b_bias):
    out, _ = kernel_run(
        dict(
            x=x,
            z=z,
            W_kernel=W_kernel,
            W_bias=W_bias,
            b_kernel=b_kernel,
            b_bias=b_bias,
        ),
        trace=False,
    )
    return out
